# revision 31
# baseline (speedup 1.0000x reference)
"""AttnReadout kernel for Trainium2, 8 NeuronCores, data-parallel over batch.

Math (per batch b, head i):
  c[i,e]    = bu[i,e] + sum_d Wv[i,e,d] * x[b, i, last_nodes[b,i], d]
  z[t,e]    = sum_d x[b,t,d] * Wu[i,e,d]          (t over O*N = 8192 tokens)
  s[t,e]    = sigmoid(z[t,e] + c[i,e])
  score[t]  = sum_e We[i,e] * s[t,e]
  alpha     = softmax(score)
  out[b,i]  = sum_t alpha[t] * x[b,t,:]

Key design points (arrived at via perfetto-trace iteration; baseline for
this problem measured ~112us locally, this kernel ~82us):
  - sigmoid(v) = (1 + tanh(v/2))/2 and the We-dot is linear, so
    score = sum_e (We_e/2)*tanh((z_e + c_e)/2) + const; softmax is
    shift-invariant so the const is dropped entirely (no exp bias).
    tanh+exp live in one ACT table set -> no table reloads.
  - The ACT (ScalarE) stream is the binding floor: 64 groups x
    (1024+171)cyc/1.2GHz ~ 64us.  ~19% of the tanh groups are computed
    instead on the otherwise-idle vector engine as an odd quintic
    u*(C1 + w*(C3 + C5*w)), w=u^2 (bf16, 2x/4x DVE modes), keeping the
    ACT, DVE and PE streams all in the low-50s us.
  - Scores land token-on-partition via s-chunk-stationary matmuls
    (LDWEIGHTS pipelines at ~25ns/chunk on HW), so exp is one cheap
    unbiased [128, 128] ACT call per sample and alpha feeds the
    weighted sum directly as a 2-column stationary.
  - The weighted sum runs as 4-way column-tiled quads (tile_position
    col groups 0/32/64/96, concurrent in the PE array, ~14ns/chunk),
    interleaved 2 quads/slot behind the main pipeline; partials are
    dumped raw and combined/normalized on the host.
  - The wsum moving operand carries a 1.0 column so the softmax
    denominator Z accumulates as output column 128 for free.
  - xt (projection side) is fp8e4m3: moving-operand precision only
    affects scores (~1e-3 end-to-end), and it halves the projection
    layout's HBM+SBUF footprint.  The wsum side (xn) stays bf16 since
    it directly forms the output values.
  - per-(b,i) tanh bias c/2 is computed on the host (last_nodes is
    host data); constants are packed into two DMA blobs (each DMA
    trigger costs ~0.6us of serial queue time at startup).
  - Prefetch DMAs for later samples are gated by a 1-element memset on
    the DVE stream (WAW dependency), so their transfers cannot start
    early and steal HBM bandwidth from earlier samples' critical loads.

Pipeline: 64 slots of (sample b, head i, 1024-token group g):
  proj(k) 2 matmuls -> [tanh(k) on ACT | quintic(k) on DVE] ->
  wedot(k-2|k-4) -> exp per sample -> wsum quads -> raw dump.
"""

import numpy as np
import ml_dtypes

import concourse.bacc as bacc
import concourse.tile as tile
from concourse import mybir
from concourse.bass_utils import run_bass_kernel_spmd

BF = ml_dtypes.bfloat16
F8 = ml_dtypes.float8_e4m3
B, O, N, D = 32, 2, 4096, 128
NCORES = 8
BPC = B // NCORES          # samples per core
T = O * N                  # tokens per sample
CH = 512                   # proj matmul moving width
G = 1024                   # tokens per pipeline slot
NG = T // G                # 8 groups per head
NC = T // 128              # 64 wsum chunks of 128 tokens
XNW = 130                  # xn chunk row width: 128 d + 1.0 + pad
CBW = O * D + O            # packed bf16 consts: wu | we
CFW = O * BPC              # packed f32 consts: ch


def _build_program():
    nc = bacc.Bacc("TRN2", target_bir_lowering=False)
    dt = mybir.dt
    f32, bf16, f8 = dt.float32, dt.bfloat16, dt.float8e4

    xt_d = nc.dram_tensor("xt", [BPC, D, T], f8, kind="ExternalInput")
    xn_d = nc.dram_tensor("xn", [BPC, D, NC, XNW], bf16, kind="ExternalInput")
    cb_d = nc.dram_tensor("cb", [D, CBW], bf16, kind="ExternalInput")
    cf_d = nc.dram_tensor("cf", [D, CFW], f32, kind="ExternalInput")
    out_d = nc.dram_tensor("out", [BPC, D, XNW], f32, kind="ExternalOutput")

    Tanh = mybir.ActivationFunctionType.Tanh
    Exp = mybir.ActivationFunctionType.Exp

    with tile.TileContext(nc) as tc:
        from contextlib import ExitStack

        with ExitStack() as ctx:
            singles = ctx.enter_context(tc.tile_pool(name="singles", bufs=1))
            sp = ctx.enter_context(tc.tile_pool(name="sp", bufs=7))
            zp = ctx.enter_context(tc.tile_pool(name="zp", bufs=2, space="PSUM"))
            zd = ctx.enter_context(tc.tile_pool(name="zd", bufs=1, space="PSUM"))
            pu = ctx.enter_context(tc.tile_pool(name="pu", bufs=2))
            pw = ctx.enter_context(tc.tile_pool(name="pw", bufs=2))
            pq = ctx.enter_context(tc.tile_pool(name="pq", bufs=2))
            scp = ctx.enter_context(tc.tile_pool(name="scp", bufs=1, space="PSUM"))
            up = ctx.enter_context(tc.tile_pool(name="up", bufs=1, space="PSUM"))
            ap = ctx.enter_context(tc.tile_pool(name="ap", bufs=3))
            smalls = ctx.enter_context(tc.tile_pool(name="smalls", bufs=2))

            # --- packed constants (one trigger per ring so both land early)
            cb_sb = singles.tile([D, CBW], bf16)
            nc.sync.dma_start(out=cb_sb, in_=cb_d[:])
            cf_sb = singles.tile([D, CFW], f32)
            nc.scalar.dma_start(out=cf_sb, in_=cf_d[:])
            wu_sb = cb_sb[:, 0 : O * D].rearrange("p (i e) -> p i e", i=O)
            we_sb = cb_sb[:, O * D : O * D + O]
            ch_sb = cf_sb[:, 0 : O * BPC]

            # --- bulk x: one big SBUF tensor per layout, sliced DMAs
            xt_sb = singles.tile([D, BPC, T], f8)
            xn_sb = singles.tile([D, BPC, NC, XNW], bf16)

            def load_xt(b, bounds, gate=False):
                if gate:
                    # 1-element memset creates a WAW dependency: the DMA
                    # then waits for this point in the DVE stream, so the
                    # transfer cannot start earlier and steal HBM bandwidth
                    # from earlier samples' still-critical loads.
                    nc.vector.memset(xt_sb[:, b, 0:1], 0.0)
                for lo, hi in zip(bounds[:-1], bounds[1:]):
                    nc.sync.dma_start(out=xt_sb[:, b, lo:hi], in_=xt_d[b, :, lo:hi])

            def load_xn(b, gate=False):
                if gate:
                    nc.vector.memset(xn_sb[:, b, 0, 0:1], 0.0)
                nc.sync.dma_start(out=xn_sb[:, b], in_=xn_d[b])

            # first sample: criticals first on the sync ring, one slice on
            # the scalar ring (ScalarE is idle until its first tanh)
            load_xt(0, [0, 1024])
            nc.scalar.dma_start(out=xt_sb[:, 0, 1024:2048], in_=xt_d[0, :, 1024:2048])
            load_xt(0, [2048, 3072, 4096, 6144, T])
            load_xn(0)

            # --- software-pipelined main loop.  Slots in DVE_SLOTS compute
            # tanh as an odd quintic on the vector engine instead of
            # ScalarE; their wedot is deferred 4 slots instead of 2 because
            # the DVE chain takes ~3.3 slots.
            slots = [
                (b, i, g) for b in range(BPC) for i in range(O) for g in range(NG)
            ]
            DVE_SLOTS = {
                k for k in range(len(slots)) if k % 4 == 3 and k >= 3 and k % 16 < 14
            }
            C1, C3, C5 = 0.94363, -0.16762, 0.01242
            MUL, ADD = mybir.AluOpType.mult, mybir.AluOpType.add
            pending = []          # [(k0, b, i, g, s_tile, depth)]
            wsum_q = []           # [(b, chunk)] ready weighted-sum chunks
            scu = {}              # per-sample score PSUM tile
            u4 = {}               # per-sample col-tiled wsum partials (PSUM)
            alpha = {}            # per-sample alpha SBUF tile
            ndone = {}            # per-sample emitted-wedot count

            def emit_wedot(bb, ii, gg, s_flat):
                for sub in range(G // D):
                    col = gg * (G // D) + sub
                    nc.tensor.matmul(
                        scu[bb][:, ii, col : col + 1],
                        s_flat[:, sub * D : (sub + 1) * D],
                        we_sb[:, ii : ii + 1],
                        start=True,
                        stop=True,
                    )
                ndone[bb] += 1
                if ndone[bb] == O * NG:
                    # all scores for sample bb are in -> one unbiased exp
                    nc.scalar.activation(
                        out=alpha[bb].rearrange("p i c -> p (i c)"),
                        in_=scu[bb].rearrange("p i c -> p (i c)"),
                        func=Exp,
                    )
                    u4[bb] = up.tile([D, XNW], f32, tag="u", name=f"u4_{bb}")
                    wsum_q.extend((bb, c) for c in range(NC))

            def emit_wsum_quad():
                for _ in range(4):
                    bb, c = wsum_q.pop(0)
                    j = c % 4
                    nc.tensor.matmul(
                        u4[bb][32 * j : 32 * j + O, 0:129],
                        alpha[bb][:, :, c],
                        xn_sb[:, bb, c, 0:129],
                        start=(c < 4),
                        stop=(c >= NC - 4),
                        tile_position=(0, 32 * j),
                    )
                if c == NC - 1:
                    # dump the 4 col-group partials raw; combine + normalize
                    # happen on the host (microseconds of numpy)
                    u_sb = smalls.tile([D, XNW], f32, tag="usb", name=f"usb{bb}")
                    nc.vector.tensor_copy(out=u_sb, in_=u4[bb])
                    nc.sync.dma_start(out=out_d[bb], in_=u_sb)

            prefetch = {
                6: lambda: load_xt(1, [0, T], gate=True),
                9: lambda: load_xn(1, gate=True),
                18: lambda: load_xt(2, [0, T], gate=True),
                22: lambda: load_xn(2, gate=True),
                34: lambda: load_xt(3, [0, T], gate=True),
                38: lambda: load_xn(3, gate=True),
            }

            for k, (b, i, g) in enumerate(slots):
                if k in prefetch:
                    prefetch[k]()
                if i == 0 and g == 0:
                    scu[b] = scp.tile(
                        [D, O, NG * (G // D)], f32, tag="scu", name=f"scu{b}"
                    )
                    alpha[b] = ap.tile([D, O, NC], bf16, tag="alpha", name=f"al{b}")
                    ndone[b] = 0
                is_dve = k in DVE_SLOTS
                # proj(k): 2 matmuls into a fresh z tile
                z_ps = (zd if is_dve else zp).tile(
                    [D, O, CH], f32, tag="zd" if is_dve else "z"
                )
                for h in range(2):
                    nc.tensor.matmul(
                        z_ps[:, h, :],
                        wu_sb[:, i, :],
                        xt_sb[:, b, g * G + h * CH : g * G + (h + 1) * CH],
                        start=True,
                        stop=True,
                    )
                # wsum: drain ready chunk quads
                for _ in range(2):
                    if wsum_q:
                        emit_wsum_quad()
                # wedots whose source tile is complete (ACT: 2 slots back,
                # DVE: 4); cap 2 per slot to bound the PE spike
                nemit = 0
                for e in list(pending):
                    if k - e[0] >= e[5] and nemit < 2:
                        pending.remove(e)
                        emit_wedot(*e[1:5])
                        nemit += 1
                # tanh(k) -> ScalarE table lookup or DVE quintic
                s_tile = sp.tile([D, G], bf16, tag="s")
                zf = z_ps.rearrange("p a b -> p (a b)")
                jcol = i * BPC + b
                if not is_dve:
                    nc.scalar.activation(
                        out=s_tile,
                        in_=zf,
                        func=Tanh,
                        bias=ch_sb[:, jcol : jcol + 1],
                    )
                    pending.append((k, b, i, g, s_tile, 2))
                else:
                    # tanh(u) ~ u*(C1 + w*(C3 + C5*w)), w = u^2, all bf16
                    u_t = pu.tile([D, G], bf16, tag="u")
                    nc.vector.tensor_scalar_add(u_t, zf, ch_sb[:, jcol : jcol + 1])
                    w_t = pw.tile([D, G], bf16, tag="w")
                    nc.vector.tensor_mul(w_t, u_t, u_t)
                    p_t = pq.tile([D, G], bf16, tag="p")
                    nc.vector.tensor_scalar(p_t, w_t, C5, C3, op0=MUL, op1=ADD)
                    q_t = pq.tile([D, G], bf16, tag="q")
                    nc.vector.tensor_mul(q_t, p_t, w_t)
                    r_t = pu.tile([D, G], bf16, tag="r")
                    nc.vector.tensor_scalar_add(r_t, q_t, C1)
                    nc.vector.tensor_mul(s_tile, r_t, u_t)
                    pending.append((k, b, i, g, s_tile, 4))

            # tail: flush remaining wedots and wsum chunks
            while pending:
                e = pending.pop(0)
                emit_wedot(*e[1:5])
            while wsum_q:
                emit_wsum_quad()

    nc.compile()
    return nc


def _prep_core_inputs(x, Wu, bu, Wv, We, last_nodes):
    """Host-side marshalling: dtype cast + layouts (weights pre-halved for
    the tanh formulation); per-(b,i) tanh bias computed here in f32."""
    x = np.ascontiguousarray(x, dtype=np.float32)
    ln = np.asarray(last_nodes).astype(np.int64)
    xb = x.reshape(B, T, D)
    xbf = xb.astype(BF)                                  # [B, T, D] bf16
    xt = np.ascontiguousarray(xb.transpose(0, 2, 1).astype(F8))  # [B, D, T] fp8
    # natural-chunked layout with a ones column:
    # xn[b, p, c, j] = x[b, c*128+p, j] (j<128); 1.0 at j=128; pad j=129
    xn = np.zeros((B, D, NC, XNW), dtype=BF)
    xn[:, :, :, :D] = xbf.reshape(B, NC, D, D).transpose(0, 2, 1, 3)
    xn[:, :, :, D] = np.array(1.0, dtype=BF)
    # tanh bias ch[e, j] = (Wv_i x_last + bu_i)[e]/2, j = i*BPC + b_local
    xl = xb[np.arange(B)[:, None], ln + np.arange(O)[None, :] * N]   # [B, O, D]
    c_half = 0.5 * (np.einsum("ied,bid->bie", Wv, xl) + bu[None])    # [B, O, D]
    wuT = (Wu * 0.5).transpose(2, 0, 1).reshape(D, O * D)            # [d, i*D+e]
    we2 = (We * 0.5).T                                               # [e, i]
    cb = np.ascontiguousarray(np.concatenate([wuT, we2], axis=1).astype(BF))

    maps = []
    for core in range(NCORES):
        sl = slice(core * BPC, (core + 1) * BPC)
        cf = np.ascontiguousarray(
            c_half[sl].transpose(2, 1, 0).reshape(D, O * BPC).astype(np.float32)
        )
        maps.append({"xt": xt[sl], "xn": xn[sl], "cb": cb, "cf": cf})
    return maps


_CACHE = {}
TRACE = False


def kernel(**inputs):
    x = np.asarray(inputs["x"])
    Wu = np.asarray(inputs["Wu"], dtype=np.float32)
    bu = np.asarray(inputs["bu"], dtype=np.float32)
    Wv = np.asarray(inputs["Wv"], dtype=np.float32)
    We = np.asarray(inputs["We"], dtype=np.float32)
    last_nodes = np.asarray(inputs["last_nodes"])

    maps = _prep_core_inputs(x, Wu, bu, Wv, We, last_nodes)
    if "nc" not in _CACHE:
        _CACHE["nc"] = _build_program()
    nc = _CACHE["nc"]
    res = run_bass_kernel_spmd(nc, maps, list(range(NCORES)), trace=TRACE)
    _CACHE["last_res"] = res
    outs = []
    for r in res.results:
        u4 = np.asarray(r["out"], dtype=np.float32)      # [BPC, D, XNW]
        part = u4.reshape(BPC, 4, 32, XNW)[:, :, :O, :]  # rows 32j+m
        u = part.sum(axis=1)                             # [BPC, O, XNW]
        outs.append(u[:, :, :D] / u[:, :, D : D + 1])
    return np.concatenate(outs, axis=0)  # [B, O, D]


if __name__ == "__main__":
    rng = np.random.default_rng(0)
    x = rng.standard_normal((B, O, N, D), dtype=np.float32)
    Wu = rng.standard_normal((O, D, D), dtype=np.float32) * 0.09
    bu = np.zeros((O, D), np.float32)
    Wv = rng.standard_normal((O, D, D), dtype=np.float32) * 0.09
    We = rng.standard_normal((O, D), dtype=np.float32) * 0.09
    ln = rng.integers(0, N, size=(B, O)).astype(np.int64)
    out = kernel(x=x, Wu=Wu, bu=bu, Wv=Wv, We=We, last_nodes=ln)
    print(out.shape, out.dtype)
